# revision 14
# baseline (speedup 1.0000x reference)
"""Trainium2 Bass kernel for a dense transformer block (B=4,T=2048,H=16,D=64,C=1024,FF=4096).

Sharding: batch b -> core pair (2b, 2b+1). Within a pair, attention is split by
heads (8 heads/core, Megatron column-parallel QKV + row-parallel W_o), the
attention output partial sums are combined with a pair ReduceScatter, and each
core then runs the full-FF MLP on its half (1024) of the rows. Output rows are
disjoint across cores; the host just concatenates.

LayerNorm affines are folded into the following matmul weights on the host,
b_o is folded into the x_own residual input, and all weights are pre-swizzled
host-side into the [partition, k-chunk, out] layout so weight DMAs are
contiguous. QKV/attention run in bf16; the MLP runs in fp8 e4m3 DoubleRow
(2 k-chunks per matmul at 2x rate) with power-of-two weight scales (wfc*16,
wout*64) undone in the epilogues. Attention computes S^T = K @ Q^T so softmax
probabilities are already in the [k, q] layout the AV matmul needs as lhsT;
the denominator comes from a ones-column in V and is broadcast across
partitions on GpSimd. The causal mask is structural (trimmed score/AV blocks,
constant triangle multiply on the diagonal), heads are software-pipelined to
keep the PE array at max p-state, exp runs on fused 2-chunk PSUM tiles, and
the LN2/residual chain for most row-chunks runs on the vector engine during
the attention tail so the MLP can start immediately after attention.
"""

import math

import ml_dtypes
import numpy as np

P = 128
B, T, H, D = 4, 2048, 16, 64
C = H * D
FF = 4096
EPS = 1e-5
N_CORES = 8

_CACHE = {}
LAST_RESULT = None


def _build(T, C, H, D, FF, n_cores, groups, sim_safe=False):
    """Build + compile the single-core SPMD program. Returns the Bacc object."""
    from contextlib import ExitStack

    import concourse.mybir as mybir
    import concourse.tile as tile
    from concourse import bacc

    dt = mybir.dt
    AF = mybir.ActivationFunctionType
    OP = mybir.AluOpType
    PM = mybir.MatmulPerfMode

    HH = H // 2               # heads per core
    QH = HH * D               # per-core c_out for each of q,k,v
    NQH = QH // P
    NT = T // P
    T2 = T // 2               # own rows
    NT2 = T2 // P
    NC = C // P
    NF = FF // P
    NG = NF // 2              # wout DoubleRow pair groups
    SL = min(512, T)          # attention q-slice width
    NSL = T // SL
    DBLK = SL // P
    HPC = P // D              # heads per 128-partition chunk
    FCW = min(512, FF)        # wfc col-chunk width
    FO = FF // FCW
    TSW = min(512, T)         # qkv t-slice width
    CSW = min(512, C)
    NCS = C // CSW
    TS2 = min(512, T2)
    HS2 = SL // 2
    NB = 4 if NC % 4 == 0 else 1  # transposes batched per psum bank
    assert QH % P == 0 and T % SL == 0 and SL % P == 0

    nc = bacc.Bacc("TRN2", target_bir_lowering=False, debug=False,
                   num_devices=n_cores)
    gelu_af = (mybir.ActivationFunctionType.Identity if sim_safe
               else mybir.ActivationFunctionType.Gelu)

    # ---- kernel I/O (weights host-swizzled to [p, kchunk, out] layouts) ----
    x_full = nc.dram_tensor("x_full", [T, C], dt.float32, kind="ExternalInput")
    x_own = nc.dram_tensor("x_own", [T2, C], dt.float32, kind="ExternalInput")
    wq = nc.dram_tensor("wq", [P, NC * QH], dt.bfloat16, kind="ExternalInput")
    wk = nc.dram_tensor("wk", [P, NC * QH], dt.bfloat16, kind="ExternalInput")
    wv = nc.dram_tensor("wv", [P, NC * QH], dt.bfloat16, kind="ExternalInput")
    bq = nc.dram_tensor("bq", [QH], dt.float32, kind="ExternalInput")
    bk = nc.dram_tensor("bk", [QH], dt.float32, kind="ExternalInput")
    bv = nc.dram_tensor("bv", [QH], dt.float32, kind="ExternalInput")
    wo = nc.dram_tensor("wo", [P, NQH * C], dt.bfloat16, kind="ExternalInput")
    wfc = nc.dram_tensor("wfc", [P, FO * NC * FCW], dt.float8e4,
                         kind="ExternalInput")
    bfc = nc.dram_tensor("bfc", [FF], dt.float32, kind="ExternalInput")
    wout = nc.dram_tensor("wout", [P, NG * 2 * C], dt.float8e4,
                          kind="ExternalInput")
    bout = nc.dram_tensor("bout", [C], dt.float32, kind="ExternalInput")
    tri = nc.dram_tensor("tri", [P, P], dt.bfloat16, kind="ExternalInput")
    ident = nc.dram_tensor("ident", [P, P], dt.bfloat16, kind="ExternalInput")
    ident8 = nc.dram_tensor("ident8", [P, P], dt.float8e4,
                            kind="ExternalInput")
    out = nc.dram_tensor("out", [T2, C], dt.float32, kind="ExternalOutput")

    # collective bounce buffers (internal DRAM)
    r_bounce = nc.dram_tensor("r_bounce", [T, C], dt.bfloat16)
    r_own_b = nc.dram_tensor("r_own_b", [T2, C], dt.bfloat16)

    x_r = x_full.rearrange("(i p) c -> p i c", p=P)
    xo_r = x_own.rearrange("(i p) c -> p i c", p=P)
    out_r = out.rearrange("(i p) c -> p i c", p=P)
    rb_r = r_bounce.rearrange("(i p) c -> p i c", p=P)
    rob_r = r_own_b.rearrange("(i p) c -> p i c", p=P)
    wfc_r = wfc.rearrange("p (fo ci o) -> p fo ci o", fo=FO, ci=NC)
    wout_r = wout.rearrange("p (g two c) -> p g two c", g=NG, two=2)

    with tile.TileContext(nc) as tc, ExitStack() as stk:
        pool_const = stk.enter_context(tc.tile_pool(name="const", bufs=1))

        tri_sb = pool_const.tile([P, P], dt.bfloat16)
        id_sb = pool_const.tile([P, P], dt.bfloat16)
        id8_sb = pool_const.tile([P, P], dt.float8e4)
        nc.sync.dma_start(tri_sb[:], tri[:])
        nc.sync.dma_start(id_sb[:], ident[:])
        nc.sync.dma_start(id8_sb[:], ident8[:])
        bq_sb = pool_const.tile([P, NQH], dt.float32)
        bk_sb = pool_const.tile([P, NQH], dt.float32)
        bv_row = pool_const.tile([1, QH], dt.float32)
        bfc_sb = pool_const.tile([P, NF], dt.float32)
        bout_row = pool_const.tile([1, C], dt.float32)
        eps_sb = pool_const.tile([P, 1], dt.float32)
        nc.vector.memset(eps_sb[:], EPS)
        bv_full = pool_const.tile([P, QH], dt.float32)
        bout_full = pool_const.tile([P, C], dt.float32)
        nc.sync.dma_start(bq_sb[:], bq.rearrange("(a p) -> p a", p=P))
        nc.sync.dma_start(bk_sb[:], bk.rearrange("(a p) -> p a", p=P))
        nc.sync.dma_start(bv_row[:], bv[None, :])
        nc.sync.dma_start(bfc_sb[:], bfc.rearrange("(a p) -> p a", p=P))
        nc.sync.dma_start(bout_row[:], bout[None, :])
        nc.gpsimd.partition_broadcast(bv_full[:], bv_row[:])
        nc.gpsimd.partition_broadcast(bout_full[:], bout_row[:])

        def ln_alloc(pool, n_chunks, nm):
            tiles = {}
            for t in ("s1", "s2", "mean", "var", "rstd", "nmr"):
                tiles[t] = pool.tile([P, n_chunks], dt.float32,
                                     tag=f"ln_{t}", name=f"{t}_{nm}")
            return tiles

        def ln_post(st, i):
            # mean/var -> rstd and -mean*rstd, given s1/s2 already filled
            ii = slice(i, i + 1)
            nc.vector.tensor_scalar_mul(st["mean"][:, ii], st["s1"][:, ii],
                                        1.0 / C)
            nc.vector.tensor_scalar_mul(st["var"][:, ii], st["s2"][:, ii],
                                        1.0 / C)
            nc.vector.tensor_tensor(st["nmr"][:, ii], st["mean"][:, ii],
                                    st["mean"][:, ii], OP.mult)
            nc.vector.tensor_tensor(st["var"][:, ii], st["var"][:, ii],
                                    st["nmr"][:, ii], OP.subtract)
            nc.scalar.activation(st["var"][:, ii], st["var"][:, ii], AF.Sqrt,
                                 bias=eps_sb[:])
            nc.vector.reciprocal_approx_fast(st["rstd"][:, ii],
                                             st["var"][:, ii])
            nc.vector.tensor_tensor(st["nmr"][:, ii], st["mean"][:, ii],
                                    st["rstd"][:, ii], OP.mult)
            nc.vector.tensor_scalar_mul(st["nmr"][:, ii], st["nmr"][:, ii],
                                        -1.0)

        def ln_chunk(st, i, xc):
            nc.vector.reduce_sum(st["s1"][:, i : i + 1], xc,
                                 axis=mybir.AxisListType.X)
            ln_post(st, i)

        # phase-4 persistent state opens before pattn (strict LIFO pools:
        # it must outlive the attention pool)
        px2 = stk.enter_context(tc.tile_pool(name="px2", bufs=1))
        X2 = px2.tile([P, NT2, C], dt.float32, tag="x2")
        Z2C = px2.tile([P, NT2, C], dt.bfloat16, tag="z2c")
        st2 = ln_alloc(px2, NT2, "ln2")
        pxo = stk.enter_context(tc.tile_pool(name="pxo", bufs=3))
        rocs = {}

        def ph4_pre(i):
            nc.sync.dma_start(X2[:, i, :], xo_r[:, i, :])
            roc = pxo.tile([P, C], dt.bfloat16, tag="roc")
            rocs[i] = roc
            nc.sync.dma_start(roc[:], rob_r[:, i, :])

        def ph4_main(i):
            nc.vector.scalar_tensor_tensor(
                X2[:, i, :], X2[:, i, :], 1.0, rocs[i][:],
                OP.bypass, OP.add, accum_out=st2["s1"][:, i : i + 1])
            sqj = pxo.tile([P, C], dt.bfloat16, tag="sqj")
            nc.vector.scalar_tensor_tensor(
                sqj[:], X2[:, i, :], 1.0, X2[:, i, :],
                OP.bypass, OP.mult, accum_out=st2["s2"][:, i : i + 1])
            ln_post(st2, i)
            nc.vector.tensor_scalar(
                Z2C[:, i, :], X2[:, i, :], st2["rstd"][:, i : i + 1],
                st2["nmr"][:, i : i + 1], OP.mult, OP.add)
            # fold b_out into the residual
            nc.vector.scalar_tensor_tensor(
                X2[:, i, :], X2[:, i, :], 1.0, bout_full[:],
                OP.bypass, OP.add)

        with tc.tile_pool(name="pattn", bufs=1) as pool_attn:
            QT = pool_attn.tile([P, NQH, T], dt.bfloat16, tag="QT")
            KT = pool_attn.tile([P, NQH, T], dt.bfloat16, tag="KT")
            V = pool_attn.tile([P, NT, HH, D + 1], dt.bfloat16, tag="V")
            YT = pool_attn.tile([P, NQH, T], dt.bfloat16, tag="YT")
            wo_sb = pool_attn.tile([P, NQH, C], dt.bfloat16, tag="wo")
            nc.vector.memset(V[:, :, :, D], 1.0)

            with ExitStack() as es_zt:
                pool_zt = es_zt.enter_context(tc.tile_pool(name="pzt", bufs=2))
                pool_wqkv = es_zt.enter_context(tc.tile_pool(name="pw1",
                                                             bufs=1))
                wq_sb = pool_wqkv.tile([P, NC, QH], dt.bfloat16, tag="wq")
                wk_sb = pool_wqkv.tile([P, NC, QH], dt.bfloat16, tag="wk")
                wv_sb = pool_wqkv.tile([P, NC, QH], dt.bfloat16, tag="wv")

                # ===== merged phase 0+1: stream x, LN1, z^T, QKV per slice ==
                with tc.tile_pool(name="pstat", bufs=1) as pool_stat, \
                     tc.tile_pool(name="pxs", bufs=2) as pool_xs, \
                     tc.tile_pool(name="ps_tra", bufs=2, space="PSUM") as ps_tra, \
                     tc.tile_pool(name="ps_mm1", bufs=4, space="PSUM") as ps_mm1:
                    st1 = ln_alloc(pool_stat, NT, "ln1")
                    ZTs = None
                    xgs = {}
                    for i in (0, 1):    # head-start the first x chunks
                        xg = pool_xs.tile([P, C], dt.float32, tag="xg",
                                          name=f"xg_pre{i}")
                        xgs[i] = xg
                        nc.sync.dma_start(xg[:], x_r[:, i, :])
                    for i in range(NT):
                        if i % 4 == 0:
                            ZTs = pool_zt.tile([P, NC, TSW], dt.bfloat16,
                                               tag="zt")
                        if i in xgs:
                            xg = xgs.pop(i)
                        else:
                            xg = pool_xs.tile([P, C], dt.float32, tag="xg")
                            nc.sync.dma_start(xg[:], x_r[:, i, :])
                        if i == 1:      # weights needed from i==3 onward
                            nc.sync.dma_start(
                                wq_sb[:],
                                wq.rearrange("p (ci o) -> p ci o", ci=NC))
                            nc.sync.dma_start(
                                wk_sb[:],
                                wk.rearrange("p (ci o) -> p ci o", ci=NC))
                            nc.sync.dma_start(
                                wv_sb[:],
                                wv.rearrange("p (ci o) -> p ci o", ci=NC))
                        xc = xg[:]
                        sq = pool_xs.tile([P, C], dt.bfloat16, tag="sq")
                        nc.scalar.activation(sq[:], xc, AF.Square,
                                             accum_out=st1["s2"][:, i : i + 1])
                        ln_chunk(st1, i, xc)
                        zc = pool_xs.tile([P, C], dt.bfloat16, tag="zc")
                        nc.scalar.activation(zc[:], xc, AF.Identity,
                                             bias=st1["nmr"][:, i : i + 1],
                                             scale=st1["rstd"][:, i : i + 1])
                        i4 = i % 4
                        for jj in range(NC // NB):
                            pt = ps_tra.tile([P, NB * P], dt.bfloat16,
                                             tag="trp")
                            for j4 in range(NB):
                                j = jj * NB + j4
                                nc.tensor.transpose(
                                    pt[:, j4 * P : (j4 + 1) * P],
                                    zc[:, j * P : (j + 1) * P], id_sb[:])
                            nc.vector.tensor_copy(
                                ZTs[:, jj * NB : (jj + 1) * NB,
                                    i4 * P : (i4 + 1) * P],
                                pt[:].rearrange("p (a b) -> p a b", a=NB))
                        if i % 4 == 3:
                            ts_ = i // 4
                            tsl = slice(ts_ * TSW, (ts_ + 1) * TSW)
                            for w_sb, dstT, b_sb in ((wq_sb, QT, bq_sb),
                                                     (wk_sb, KT, bk_sb)):
                                for co in range(NQH):
                                    pm = ps_mm1.tile([P, TSW], dt.float32,
                                                     tag="mmp")
                                    for ci in range(NC):
                                        nc.tensor.matmul(
                                            pm[:],
                                            w_sb[:, ci, co * P : (co + 1) * P],
                                            ZTs[:, ci, :],
                                            start=(ci == 0),
                                            stop=(ci == NC - 1))
                                    nc.vector.tensor_scalar(
                                        dstT[:, co, tsl], pm[:],
                                        b_sb[:, co : co + 1], None, OP.add)
                            for tis in range(4):
                                ti = 4 * ts_ + tis
                                pm = ps_mm1.tile([P, QH], dt.float32,
                                                 tag="mmp")
                                for ci in range(NC):
                                    nc.tensor.matmul(
                                        pm[:],
                                        ZTs[:, ci, tis * P : (tis + 1) * P],
                                        wv_sb[:, ci, :],
                                        start=(ci == 0), stop=(ci == NC - 1))
                                nc.vector.tensor_tensor(
                                    V[:, ti, :, :D],
                                    pm[:].rearrange("p (h d) -> p h d", d=D),
                                    bv_full[:].rearrange("p (h d) -> p h d",
                                                         d=D),
                                    OP.add)

            # ===== attention: head-pipelined scores/AV + W_o + chunked RS ====
            inv_sqrt_d = 1.0 / math.sqrt(D)
            nc.sync.dma_start(wo_sb[:],
                              wo.rearrange("p (ci o) -> p ci o", ci=NQH))
            TPS = SL // P      # t-chunks per q-slice
            with tc.tile_pool(name="ppt", bufs=2) as pool_pt, \
                 tc.tile_pool(name="prec", bufs=2) as pool_rec, \
                 tc.tile_pool(name="prs", bufs=3) as pool_rs, \
                 tc.tile_pool(name="ps_s", bufs=2, space="PSUM") as ps_s, \
                 tc.tile_pool(name="ps_o", bufs=2, space="PSUM") as ps_o, \
                 tc.tile_pool(name="ps_w", bufs=2, space="PSUM") as ps_w:

                def s_pair(s, h, a, PT_h):
                    """Two score matmuls into one 2-bank PSUM tile + one exp."""
                    hc, hp = h // HPC, D * (h % HPC)
                    kc0 = 2 * a
                    pm2 = ps_s.tile([P, 2 * SL], dt.float32, tag="sp2")
                    for idx in (0, 1):
                        kc = kc0 + idx
                        c0 = max(kc - s * DBLK, 0) * P
                        nc.tensor.matmul(
                            pm2[:, idx * SL + c0 : (idx + 1) * SL],
                            KT[hp : hp + D, hc, kc * P : (kc + 1) * P],
                            QT[hp : hp + D, hc,
                               s * SL + c0 : (s + 1) * SL],
                            start=True, stop=True)
                    c0e = max(kc0 - s * DBLK, 0) * P
                    ptf = PT_h[:].rearrange("p a b -> p (a b)")
                    nc.scalar.activation(
                        ptf[:, kc0 * SL + c0e : (kc0 + 2) * SL],
                        pm2[:, c0e:], AF.Exp, scale=inv_sqrt_d)
                    for idx in (0, 1):
                        kc = kc0 + idx
                        j = kc - s * DBLK
                        if j >= 0:
                            nc.vector.tensor_tensor(
                                PT_h[:, kc, j * P : (j + 1) * P],
                                PT_h[:, kc, j * P : (j + 1) * P],
                                tri_sb[:], OP.mult)

                def av_chunk(sp, hp_, po, PT_p, kc, kcm):
                    c0 = max(kc - sp * DBLK, 0) * P
                    nc.tensor.matmul(po[: D + 1, c0:],
                                     V[:, kc, hp_, :], PT_p[:, kc, c0:],
                                     start=(kc == 0), stop=(kc == kcm - 1),
                                     skip_group_check=True)

                def den_yt(sp, hp_, po):
                    hc, hpp = hp_ // HPC, D * (hp_ % HPC)
                    dcp = pool_rec.tile([1, SL], dt.float32, tag="dcp")
                    nc.vector.tensor_copy(dcp[:], po[D : D + 1, :])
                    den = pool_rec.tile([1, SL], dt.float32, tag="den")
                    nc.vector.reciprocal_approx_fast(den[:], dcp[:])
                    recb = pool_rec.tile([D, SL], dt.float32, tag="recb")
                    nc.gpsimd.partition_broadcast(recb[:], den[:])
                    nc.vector.tensor_tensor(
                        YT[hpp : hpp + D, hc, sp * SL : (sp + 1) * SL],
                        po[:D, :], recb[:], OP.mult)

                def wo_slice(s, order=None, blkmap=None):
                    for tis in (order or range(TPS)):
                        ti = s * TPS + tis
                        blk = blkmap.get(ti, ti) if blkmap else ti
                        r_sb = pool_rs.tile([P, C], dt.bfloat16, tag="rsb")
                        for cs in range(NCS):
                            pm = ps_w.tile([P, CSW], dt.float32, tag="wop")
                            for ci in range(NQH):
                                nc.tensor.matmul(
                                    pm[:],
                                    YT[:, ci, ti * P : (ti + 1) * P],
                                    wo_sb[:, ci, cs * CSW : (cs + 1) * CSW],
                                    start=(ci == 0), stop=(ci == NQH - 1))
                            nc.vector.tensor_copy(
                                r_sb[:, cs * CSW : (cs + 1) * CSW], pm[:])
                        nc.sync.dma_start(rb_r[:, blk, :], r_sb[:])

                def rs_slice(s):
                    nc.gpsimd.collective_compute(
                        "ReduceScatter", OP.add, replica_groups=groups,
                        ins=[r_bounce[s * SL : (s + 1) * SL, :].opt()],
                        outs=[r_own_b[s * HS2 : (s + 1) * HS2, :].opt()])

                def rs_half(s, q):
                    # 256-row RS over a contiguous bounce range; wo_slice's
                    # blkmap placed each core's own rows in the right half
                    nc.gpsimd.collective_compute(
                        "ReduceScatter", OP.add, replica_groups=groups,
                        ins=[r_bounce[s * SL + q * 2 * P :
                                      s * SL + (q + 1) * 2 * P, :].opt()],
                        outs=[r_own_b[s * HS2 + q * P :
                                      s * HS2 + (q + 1) * P, :].opt()])

                pend = None   # (s, h, PT_h, kcm) of the head awaiting AV
                for s in range(NSL):
                    kcm = (s + 1) * DBLK
                    for h in range(HH):
                        PT_h = pool_pt.tile([P, NT, SL], dt.bfloat16,
                                            tag="PT")
                        npairs = kcm // 2
                        po = None
                        av_i = 0
                        if pend is not None:
                            ps_, ph_, pPT, pkcm = pend
                            per = -(-pkcm // npairs)
                        for a in range(npairs):
                            s_pair(s, h, a, PT_h)
                            if pend is not None:
                                for _ in range(per):
                                    if av_i < pkcm:
                                        if po is None:
                                            po = ps_o.tile([P, SL],
                                                           dt.float32,
                                                           tag="op")
                                        av_chunk(ps_, ph_, po, pPT,
                                                 av_i, pkcm)
                                        av_i += 1
                        if pend is not None:
                            while av_i < pkcm:
                                av_chunk(ps_, ph_, po, pPT, av_i, pkcm)
                                av_i += 1
                            den_yt(ps_, ph_, po)
                        pend = (s, h, PT_h, kcm)
                        if h == 0 and s > 0:
                            wo_slice(s - 1)
                        if h == 3 and s > 0:
                            rs_slice(s - 1)
                        # overlap phase-4 residual+LN2 into the last slice
                        if s == NSL - 1:
                            if h < NT2 - 2:
                                ph4_pre(h)
                            if h >= 2:
                                ph4_main(h - 2)
                # drain last head + last slice W_o / RS
                ps_, ph_, pPT, pkcm = pend
                po = ps_o.tile([P, SL], dt.float32, tag="op")
                for kc in range(pkcm):
                    av_chunk(ps_, ph_, po, pPT, kc, pkcm)
                den_yt(ps_, ph_, po)
                t0 = (NSL - 1) * TPS
                wo_slice(NSL - 1, order=(0, 2, 1, 3),
                         blkmap={t0: t0, t0 + 2: t0 + 1,
                                 t0 + 1: t0 + 2, t0 + 3: t0 + 3})
                rs_half(NSL - 1, 0)
                rs_half(NSL - 1, 1)

        # ===== phase 4 tail + 5 =====
        with tc.tile_pool(name="pht", bufs=1) as pool_ht:
            HT = pool_ht.tile([P, NF, T2], dt.float8e4)

            with ExitStack() as es_z2t:
                pool_z2t = es_z2t.enter_context(
                    tc.tile_pool(name="pz2t", bufs=1))
                Z2T = pool_z2t.tile([P, NC, T2], dt.float8e4)
                pool_wfc = es_z2t.enter_context(
                    tc.tile_pool(name="pwfc", bufs=3))
                ps_h = es_z2t.enter_context(
                    tc.tile_pool(name="ps_h", bufs=4, space="PSUM"))
                ps_trb = es_z2t.enter_context(
                    tc.tile_pool(name="ps_trb", bufs=2, space="PSUM"))

                # chunks 6,7 (needed RS of the last slice)
                for i in (NT2 - 2, NT2 - 1):
                    ph4_pre(i)
                for i in (NT2 - 2, NT2 - 1):
                    ph4_main(i)

                for i in range(NT2):
                    for jj in range(NC // NB):
                        pt = ps_trb.tile([P, NB * P], dt.bfloat16,
                                         tag="trp")
                        for j4 in range(NB):
                            j = jj * NB + j4
                            nc.tensor.transpose(
                                pt[:, j4 * P : (j4 + 1) * P],
                                Z2C[:, i, j * P : (j + 1) * P], id_sb[:])
                        nc.vector.tensor_copy(
                            Z2T[:, jj * NB : (jj + 1) * NB,
                                i * P : (i + 1) * P],
                            pt[:].rearrange("p (a b) -> p a b", a=NB))
                    # FC + gelu (fp8 DoubleRow) for the finished 512-row slice
                    if i % 4 == 3:
                        ts_ = i // 4
                        tsl = slice(ts_ * TS2, (ts_ + 1) * TS2)
                        for fo in range(FO):
                            wfc_sb = pool_wfc.tile([P, NC, FCW],
                                                   dt.float8e4, tag="wfc")
                            nc.sync.dma_start(wfc_sb[:], wfc_r[:, fo])
                            for f in range(FCW // P):
                                fg = fo * (FCW // P) + f
                                pm = ps_h.tile([P, TS2], dt.float32,
                                               tag="hp")
                                for j in range(NC // 2):
                                    nc.tensor.matmul(
                                        pm[:],
                                        wfc_sb[:, 2 * j : 2 * j + 2,
                                               f * P : (f + 1) * P],
                                        Z2T[:, 2 * j : 2 * j + 2, tsl],
                                        start=(j == 0),
                                        stop=(j == NC // 2 - 1),
                                        perf_mode=PM.DoubleRow)
                                nc.scalar.activation(
                                    HT[:, fg, tsl],
                                    pm[:], gelu_af, scale=1.0 / 16,
                                    bias=bfc_sb[:, fg : fg + 1])
                es_z2t.close()

            # phase 5b: W_out (fp8 DoubleRow) + residual
            with tc.tile_pool(name="pwout", bufs=4) as pool_wout, \
                 tc.tile_pool(name="pout", bufs=3) as pool_out, \
                 tc.tile_pool(name="ps_out", bufs=1,
                              space="PSUM") as ps_out:
                for cs in range(NCS):
                    pms = [ps_out.tile([P, CSW], dt.float32,
                                       tag=f"outp{ti}",
                                       name=f"outp_{cs}_{ti}")
                           for ti in range(NT2)]
                    for g in range(NG):
                        wout_sb = pool_wout.tile([P, 2, CSW], dt.float8e4,
                                                 tag="wout")
                        nc.sync.dma_start(
                            wout_sb[:],
                            wout_r[:, g, :, cs * CSW : (cs + 1) * CSW])
                        for ti in range(NT2):
                            nc.tensor.matmul(
                                pms[ti][:],
                                HT[:, 2 * g : 2 * g + 2,
                                   ti * P : (ti + 1) * P],
                                wout_sb[:],
                                start=(g == 0), stop=(g == NG - 1),
                                perf_mode=PM.DoubleRow)
                    for ti in range(NT2):
                        o_sb = pool_out.tile([P, CSW], dt.float32,
                                             tag="osb")
                        nc.vector.scalar_tensor_tensor(
                            o_sb[:], pms[ti][:], 1.0 / 64,
                            X2[:, ti, cs * CSW : (cs + 1) * CSW],
                            OP.mult, OP.add)
                        nc.sync.dma_start(
                            out_r[:, ti, cs * CSW : (cs + 1) * CSW],
                            o_sb[:])

    nc.compile()
    return nc


def _prep_core_inputs(b, parity, x, ln1_w, ln1_b, w_qkv, b_qkv, w_o, b_o,
                      ln2_w, ln2_b, w_fc, b_fc, w_out, b_out,
                      T_, C_, H_, D_):
    """Host-side per-core input dict (weights LN-folded + swizzled)."""
    bf16 = ml_dtypes.bfloat16
    fp8 = ml_dtypes.float8_e4m3
    HH = H_ // 2
    QH = HH * D_
    NC_ = C_ // P
    FF_ = w_fc.shape[1]
    NF_ = FF_ // P
    NG_ = NF_ // 2
    FCW_ = min(512, FF_)
    FO_ = FF_ // FCW_
    wq_eff = (ln1_w[:, None] * w_qkv).astype(np.float32)
    bq_eff = (b_qkv + ln1_b @ w_qkv).astype(np.float32)
    wfc_eff = (ln2_w[:, None] * w_fc).astype(np.float32)
    bfc_eff = (b_fc + ln2_b @ w_fc).astype(np.float32)

    def swiz_k(w):
        # [C, O] -> [P, NC, O] (partition-contiguous k-chunks), flattened
        O = w.shape[1]
        return np.ascontiguousarray(
            w.reshape(NC_, P, O).transpose(1, 0, 2).reshape(P, NC_ * O))

    h0 = parity * QH
    sl_q = slice(h0, h0 + QH)
    sl_k = slice(C_ + h0, C_ + h0 + QH)
    sl_v = slice(2 * C_ + h0, 2 * C_ + h0 + QH)
    tri = np.tril(np.ones((P, P), np.float32)).T  # tri[k,q] = 1 if k <= q
    ident = np.eye(P, dtype=np.float32)
    SL_ = min(512, T_)
    HS = SL_ // 2
    own_rows = np.concatenate([
        np.arange(s * SL_ + parity * HS, s * SL_ + (parity + 1) * HS)
        for s in range(T_ // SL_)])
    # wfc: [P, FO, NC, FCW] so each fo-chunk DMA is partition-contiguous
    wfc8 = (wfc_eff * 16).astype(fp8)
    wfc_sw = (wfc8.reshape(NC_, P, FO_, FCW_)
              .transpose(1, 2, 0, 3).reshape(P, FO_ * NC_ * FCW_))
    # wout: [P, NG, 2, C] DoubleRow pair layout
    wout8 = (w_out * 64).astype(fp8)
    wout_sw = (wout8.reshape(NG_, 2, P, C_)
               .transpose(2, 0, 1, 3).reshape(P, NG_ * 2 * C_))
    return {
        "x_full": np.ascontiguousarray(x[b]),
        # b_o is folded into the attention residual here
        "x_own": np.ascontiguousarray(x[b, own_rows] + b_o[None, :]),
        "wq": swiz_k(wq_eff[:, sl_q].astype(bf16)),
        "wk": swiz_k(wq_eff[:, sl_k].astype(bf16)),
        "wv": swiz_k(wq_eff[:, sl_v].astype(bf16)),
        "bq": np.ascontiguousarray(bq_eff[sl_q]),
        "bk": np.ascontiguousarray(bq_eff[sl_k]),
        "bv": np.ascontiguousarray(bq_eff[sl_v]),
        "wo": np.ascontiguousarray(
            w_o[h0 : h0 + QH, :].astype(bf16).reshape(QH // P, P, C_)
            .transpose(1, 0, 2).reshape(P, (QH // P) * C_)),
        "wfc": np.ascontiguousarray(wfc_sw),
        "bfc": np.ascontiguousarray(bfc_eff),
        "wout": np.ascontiguousarray(wout_sw),
        "bout": np.ascontiguousarray(b_out),
        "tri": tri.astype(bf16),
        "ident": ident.astype(bf16),
        "ident8": ident.astype(fp8),
    }


def kernel(x, ln1_w, ln1_b, w_qkv, b_qkv, w_o, b_o, ln2_w, ln2_b,
           w_fc, b_fc, w_out, b_out):
    from concourse.bass_utils import run_bass_kernel_spmd

    key = (T, C, H, D, FF, N_CORES)
    if key not in _CACHE:
        groups = [[2 * i, 2 * i + 1] for i in range(N_CORES // 2)]
        _CACHE[key] = _build(T, C, H, D, FF, N_CORES, groups)
    nc = _CACHE[key]

    args = (np.asarray(x, np.float32), np.asarray(ln1_w, np.float32),
            np.asarray(ln1_b, np.float32), np.asarray(w_qkv, np.float32),
            np.asarray(b_qkv, np.float32), np.asarray(w_o, np.float32),
            np.asarray(b_o, np.float32), np.asarray(ln2_w, np.float32),
            np.asarray(ln2_b, np.float32), np.asarray(w_fc, np.float32),
            np.asarray(b_fc, np.float32), np.asarray(w_out, np.float32),
            np.asarray(b_out, np.float32))
    in_maps = []
    for core in range(N_CORES):
        b, parity = core // 2, core % 2
        in_maps.append(_prep_core_inputs(b, parity, *args, T, C, H, D))

    global LAST_RESULT
    res = run_bass_kernel_spmd(nc, in_maps, core_ids=list(range(N_CORES)))
    LAST_RESULT = res

    SL_ = min(512, T)
    HS = SL_ // 2
    full = np.empty((B, T, C), np.float32)
    for core in range(N_CORES):
        b, parity = core // 2, core % 2
        o = res.results[core]["out"]
        for s in range(T // SL_):
            full[b, s * SL_ + parity * HS : s * SL_ + (parity + 1) * HS] = \
                o[s * HS : (s + 1) * HS]
    return full


# revision 17
# speedup vs baseline: 1.0133x; 1.0133x over previous
"""Trainium2 Bass kernel for a dense transformer block (B=4,T=2048,H=16,D=64,C=1024,FF=4096).

Sharding: batch b -> core pair (2b, 2b+1). Within a pair, attention is split by
heads (8 heads/core, Megatron column-parallel QKV + row-parallel W_o), the
attention output partial sums are combined with a pair ReduceScatter, and each
core then runs the full-FF MLP on its half (1024) of the rows. Output rows are
disjoint across cores; the host just concatenates.

LayerNorm affines are folded into the following matmul weights on the host,
b_o is folded into the x_own residual input, and all weights are pre-swizzled
host-side into the [partition, k-chunk, out] layout so weight DMAs are
contiguous. QKV/attention run in bf16; the MLP runs in fp8 e4m3 DoubleRow
(2 k-chunks per matmul at 2x rate) with power-of-two weight scales (wfc*16,
wout*64) undone in the epilogues. Attention computes S^T = K @ Q^T so softmax
probabilities are already in the [k, q] layout the AV matmul needs as lhsT;
the denominator comes from a ones-column in V and is broadcast across
partitions on GpSimd. The causal mask is structural (trimmed score/AV blocks,
constant triangle multiply on the diagonal), heads are software-pipelined to
keep the PE array at max p-state, exp runs on fused 2-chunk PSUM tiles, and
the LN2/residual chain for most row-chunks runs on the vector engine during
the attention tail so the MLP can start immediately after attention.
"""

import math

import ml_dtypes
import numpy as np

P = 128
B, T, H, D = 4, 2048, 16, 64
C = H * D
FF = 4096
EPS = 1e-5
N_CORES = 8

_CACHE = {}
LAST_RESULT = None


def _build(T, C, H, D, FF, n_cores, groups, sim_safe=False):
    """Build + compile the single-core SPMD program. Returns the Bacc object."""
    from contextlib import ExitStack

    import concourse.mybir as mybir
    import concourse.tile as tile
    from concourse import bacc

    dt = mybir.dt
    AF = mybir.ActivationFunctionType
    OP = mybir.AluOpType
    PM = mybir.MatmulPerfMode

    HH = H // 2               # heads per core
    QH = HH * D               # per-core c_out for each of q,k,v
    NQH = QH // P
    NT = T // P
    T2 = T // 2               # own rows
    NT2 = T2 // P
    NC = C // P
    NF = FF // P
    NG = NF // 2              # wout DoubleRow pair groups
    SL = min(512, T)          # attention q-slice width
    NSL = T // SL
    DBLK = SL // P
    HPC = P // D              # heads per 128-partition chunk
    FCW = min(512, FF)        # wfc col-chunk width
    FO = FF // FCW
    TSW = min(512, T)         # qkv t-slice width
    CSW = min(512, C)
    NCS = C // CSW
    TS2 = min(512, T2)
    HS2 = SL // 2
    NB = 4 if NC % 4 == 0 else 1  # transposes batched per psum bank
    assert QH % P == 0 and T % SL == 0 and SL % P == 0

    nc = bacc.Bacc("TRN2", target_bir_lowering=False, debug=False,
                   num_devices=n_cores)
    gelu_af = (mybir.ActivationFunctionType.Identity if sim_safe
               else mybir.ActivationFunctionType.Gelu)

    # ---- kernel I/O (weights host-swizzled to [p, kchunk, out] layouts) ----
    x_full = nc.dram_tensor("x_full", [T, C], dt.float32, kind="ExternalInput")
    x_own = nc.dram_tensor("x_own", [T2, C], dt.float32, kind="ExternalInput")
    wq = nc.dram_tensor("wq", [P, NC * QH], dt.bfloat16, kind="ExternalInput")
    wk = nc.dram_tensor("wk", [P, NC * QH], dt.bfloat16, kind="ExternalInput")
    wv = nc.dram_tensor("wv", [P, NC * QH], dt.bfloat16, kind="ExternalInput")
    bq = nc.dram_tensor("bq", [QH], dt.float32, kind="ExternalInput")
    bk = nc.dram_tensor("bk", [QH], dt.float32, kind="ExternalInput")
    bv = nc.dram_tensor("bv", [QH], dt.float32, kind="ExternalInput")
    wo = nc.dram_tensor("wo", [P, NQH * C], dt.bfloat16, kind="ExternalInput")
    wfc = nc.dram_tensor("wfc", [P, FO * NC * FCW], dt.float8e4,
                         kind="ExternalInput")
    bfc = nc.dram_tensor("bfc", [FF], dt.float32, kind="ExternalInput")
    wout = nc.dram_tensor("wout", [P, NG * 2 * C], dt.float8e4,
                          kind="ExternalInput")
    bout = nc.dram_tensor("bout", [C], dt.float32, kind="ExternalInput")
    tri = nc.dram_tensor("tri", [P, P], dt.bfloat16, kind="ExternalInput")
    ident = nc.dram_tensor("ident", [P, P], dt.bfloat16, kind="ExternalInput")
    ident8 = nc.dram_tensor("ident8", [P, P], dt.float8e4,
                            kind="ExternalInput")
    out = nc.dram_tensor("out", [T2, C], dt.float32, kind="ExternalOutput")

    # collective bounce buffers (internal DRAM)
    r_bounce = nc.dram_tensor("r_bounce", [T, C], dt.bfloat16)
    r_own_b = nc.dram_tensor("r_own_b", [T2, C], dt.bfloat16)

    x_r = x_full.rearrange("(i p) c -> p i c", p=P)
    xo_r = x_own.rearrange("(i p) c -> p i c", p=P)
    out_r = out.rearrange("(i p) c -> p i c", p=P)
    rb_r = r_bounce.rearrange("(i p) c -> p i c", p=P)
    rob_r = r_own_b.rearrange("(i p) c -> p i c", p=P)
    wfc_r = wfc.rearrange("p (fo ci o) -> p fo ci o", fo=FO, ci=NC)
    wout_r = wout.rearrange("p (g two c) -> p g two c", g=NG, two=2)

    with tile.TileContext(nc) as tc, ExitStack() as stk:
        pool_const = stk.enter_context(tc.tile_pool(name="const", bufs=1))

        tri_sb = pool_const.tile([P, P], dt.bfloat16)
        id_sb = pool_const.tile([P, P], dt.bfloat16)
        id8_sb = pool_const.tile([P, P], dt.float8e4)
        nc.sync.dma_start(tri_sb[:], tri[:])
        nc.sync.dma_start(id_sb[:], ident[:])
        nc.sync.dma_start(id8_sb[:], ident8[:])
        bq_sb = pool_const.tile([P, NQH], dt.float32)
        bk_sb = pool_const.tile([P, NQH], dt.float32)
        bv_row = pool_const.tile([1, QH], dt.float32)
        bfc_sb = pool_const.tile([P, NF], dt.float32)
        bout_row = pool_const.tile([1, C], dt.float32)
        eps_sb = pool_const.tile([P, 1], dt.float32)
        nc.vector.memset(eps_sb[:], EPS)
        bv_full = pool_const.tile([P, QH], dt.float32)
        bout_full = pool_const.tile([P, C], dt.float32)
        nc.sync.dma_start(bq_sb[:], bq.rearrange("(a p) -> p a", p=P))
        nc.sync.dma_start(bk_sb[:], bk.rearrange("(a p) -> p a", p=P))
        nc.sync.dma_start(bv_row[:], bv[None, :])
        nc.sync.dma_start(bfc_sb[:], bfc.rearrange("(a p) -> p a", p=P))
        nc.sync.dma_start(bout_row[:], bout[None, :])
        nc.gpsimd.partition_broadcast(bv_full[:], bv_row[:])
        nc.gpsimd.partition_broadcast(bout_full[:], bout_row[:])

        def ln_alloc(pool, n_chunks, nm):
            tiles = {}
            for t in ("s1", "s2", "mean", "var", "rstd", "nmr"):
                tiles[t] = pool.tile([P, n_chunks], dt.float32,
                                     tag=f"ln_{t}", name=f"{t}_{nm}")
            return tiles

        def ln_post(st, i):
            # mean/var -> rstd and -mean*rstd, given s1/s2 already filled
            ii = slice(i, i + 1)
            nc.vector.tensor_scalar_mul(st["mean"][:, ii], st["s1"][:, ii],
                                        1.0 / C)
            nc.vector.tensor_scalar_mul(st["var"][:, ii], st["s2"][:, ii],
                                        1.0 / C)
            nc.vector.tensor_tensor(st["nmr"][:, ii], st["mean"][:, ii],
                                    st["mean"][:, ii], OP.mult)
            nc.vector.tensor_tensor(st["var"][:, ii], st["var"][:, ii],
                                    st["nmr"][:, ii], OP.subtract)
            nc.scalar.activation(st["var"][:, ii], st["var"][:, ii], AF.Sqrt,
                                 bias=eps_sb[:])
            nc.vector.reciprocal_approx_fast(st["rstd"][:, ii],
                                             st["var"][:, ii])
            nc.vector.tensor_tensor(st["nmr"][:, ii], st["mean"][:, ii],
                                    st["rstd"][:, ii], OP.mult)
            nc.vector.tensor_scalar_mul(st["nmr"][:, ii], st["nmr"][:, ii],
                                        -1.0)

        def ln_chunk(st, i, xc):
            nc.vector.reduce_sum(st["s1"][:, i : i + 1], xc,
                                 axis=mybir.AxisListType.X)
            ln_post(st, i)

        # phase-4 persistent state opens before pattn (strict LIFO pools:
        # it must outlive the attention pool)
        px2 = stk.enter_context(tc.tile_pool(name="px2", bufs=1))
        X2 = px2.tile([P, NT2, C], dt.float32, tag="x2")
        Z2C = px2.tile([P, NT2, C], dt.bfloat16, tag="z2c")
        st2 = ln_alloc(px2, NT2, "ln2")
        pxo = stk.enter_context(tc.tile_pool(name="pxo", bufs=2))
        rocs = {}

        def ph4_pre(i):
            nc.sync.dma_start(X2[:, i, :], xo_r[:, i, :])
            roc = pxo.tile([P, C], dt.bfloat16, tag="roc")
            rocs[i] = roc
            nc.sync.dma_start(roc[:], rob_r[:, i, :])

        def ph4_main(i):
            nc.vector.scalar_tensor_tensor(
                X2[:, i, :], X2[:, i, :], 1.0, rocs[i][:],
                OP.bypass, OP.add, accum_out=st2["s1"][:, i : i + 1])
            nc.vector.scalar_tensor_tensor(
                rocs[i][:], X2[:, i, :], 1.0, X2[:, i, :],
                OP.bypass, OP.mult, accum_out=st2["s2"][:, i : i + 1])
            ln_post(st2, i)
            nc.vector.tensor_scalar(
                Z2C[:, i, :], X2[:, i, :], st2["rstd"][:, i : i + 1],
                st2["nmr"][:, i : i + 1], OP.mult, OP.add)
            # fold b_out into the residual
            nc.vector.scalar_tensor_tensor(
                X2[:, i, :], X2[:, i, :], 1.0, bout_full[:],
                OP.bypass, OP.add)

        with tc.tile_pool(name="pattn", bufs=1) as pool_attn:
            QT = pool_attn.tile([P, NQH, T], dt.bfloat16, tag="QT")
            KT = pool_attn.tile([P, NQH, T], dt.bfloat16, tag="KT")
            V = pool_attn.tile([P, NT, HH, 2 * D], dt.bfloat16, tag="V")
            YT = pool_attn.tile([P, NQH, T], dt.bfloat16, tag="YT")
            wo_sb = pool_attn.tile([P, NQH, C], dt.bfloat16, tag="wo")
            nc.vector.memset(V[:, :, :, D:], 1.0)

            with ExitStack() as es_zt:
                pool_zt = es_zt.enter_context(tc.tile_pool(name="pzt", bufs=2))
                pool_wqkv = es_zt.enter_context(tc.tile_pool(name="pw1",
                                                             bufs=1))
                wq_sb = pool_wqkv.tile([P, NC, QH], dt.bfloat16, tag="wq")
                wk_sb = pool_wqkv.tile([P, NC, QH], dt.bfloat16, tag="wk")
                wv_sb = pool_wqkv.tile([P, NC, QH], dt.bfloat16, tag="wv")

                # ===== merged phase 0+1: stream x, LN1, z^T, QKV per slice ==
                with tc.tile_pool(name="pstat", bufs=1) as pool_stat, \
                     tc.tile_pool(name="pxs", bufs=2) as pool_xs, \
                     tc.tile_pool(name="ps_tra", bufs=2, space="PSUM") as ps_tra, \
                     tc.tile_pool(name="ps_mm1", bufs=4, space="PSUM") as ps_mm1:
                    st1 = ln_alloc(pool_stat, NT, "ln1")
                    ZTs = None
                    xgs = {}
                    for i in (0, 1):    # head-start the first x chunks
                        xg = pool_xs.tile([P, C], dt.float32, tag="xg",
                                          name=f"xg_pre{i}")
                        xgs[i] = xg
                        nc.sync.dma_start(xg[:], x_r[:, i, :])
                    for i in range(NT):
                        if i % 4 == 0:
                            ZTs = pool_zt.tile([P, NC, TSW], dt.bfloat16,
                                               tag="zt")
                        if i in xgs:
                            xg = xgs.pop(i)
                        else:
                            xg = pool_xs.tile([P, C], dt.float32, tag="xg")
                            nc.sync.dma_start(xg[:], x_r[:, i, :])
                        if i == 1:      # weights needed from i==3 onward
                            nc.sync.dma_start(
                                wq_sb[:],
                                wq.rearrange("p (ci o) -> p ci o", ci=NC))
                            nc.sync.dma_start(
                                wk_sb[:],
                                wk.rearrange("p (ci o) -> p ci o", ci=NC))
                            nc.sync.dma_start(
                                wv_sb[:],
                                wv.rearrange("p (ci o) -> p ci o", ci=NC))
                        xc = xg[:]
                        zc = pool_xs.tile([P, C], dt.bfloat16, tag="zc")
                        nc.scalar.activation(zc[:], xc, AF.Square,
                                             accum_out=st1["s2"][:, i : i + 1])
                        ln_chunk(st1, i, xc)
                        nc.scalar.activation(zc[:], xc, AF.Identity,
                                             bias=st1["nmr"][:, i : i + 1],
                                             scale=st1["rstd"][:, i : i + 1])
                        i4 = i % 4
                        for jj in range(NC // NB):
                            pt = ps_tra.tile([P, NB * P], dt.bfloat16,
                                             tag="trp")
                            for j4 in range(NB):
                                j = jj * NB + j4
                                nc.tensor.transpose(
                                    pt[:, j4 * P : (j4 + 1) * P],
                                    zc[:, j * P : (j + 1) * P], id_sb[:])
                            nc.vector.tensor_copy(
                                ZTs[:, jj * NB : (jj + 1) * NB,
                                    i4 * P : (i4 + 1) * P],
                                pt[:].rearrange("p (a b) -> p a b", a=NB))
                        if i % 4 == 3:
                            ts_ = i // 4
                            tsl = slice(ts_ * TSW, (ts_ + 1) * TSW)
                            for w_sb, dstT, b_sb in ((wq_sb, QT, bq_sb),
                                                     (wk_sb, KT, bk_sb)):
                                for co in range(NQH):
                                    pm = ps_mm1.tile([P, TSW], dt.float32,
                                                     tag="mmp")
                                    for ci in range(NC):
                                        nc.tensor.matmul(
                                            pm[:],
                                            w_sb[:, ci, co * P : (co + 1) * P],
                                            ZTs[:, ci, :],
                                            start=(ci == 0),
                                            stop=(ci == NC - 1))
                                    nc.vector.tensor_scalar(
                                        dstT[:, co, tsl], pm[:],
                                        b_sb[:, co : co + 1], None, OP.add)
                            for tis in range(4):
                                ti = 4 * ts_ + tis
                                pm = ps_mm1.tile([P, QH], dt.float32,
                                                 tag="mmp")
                                for ci in range(NC):
                                    nc.tensor.matmul(
                                        pm[:],
                                        ZTs[:, ci, tis * P : (tis + 1) * P],
                                        wv_sb[:, ci, :],
                                        start=(ci == 0), stop=(ci == NC - 1))
                                nc.vector.tensor_tensor(
                                    V[:, ti, :, :D],
                                    pm[:].rearrange("p (h d) -> p h d", d=D),
                                    bv_full[:].rearrange("p (h d) -> p h d",
                                                         d=D),
                                    OP.add)

            # ===== attention: head-pipelined scores/AV + W_o + chunked RS ====
            inv_sqrt_d = 1.0 / math.sqrt(D)
            nc.sync.dma_start(wo_sb[:],
                              wo.rearrange("p (ci o) -> p ci o", ci=NQH))
            TPS = SL // P      # t-chunks per q-slice
            with tc.tile_pool(name="ppt", bufs=2) as pool_pt, \
                 tc.tile_pool(name="prec", bufs=2) as pool_rec, \
                 tc.tile_pool(name="prs", bufs=3) as pool_rs, \
                 tc.tile_pool(name="ps_s", bufs=2, space="PSUM") as ps_s, \
                 tc.tile_pool(name="ps_o", bufs=2, space="PSUM") as ps_o, \
                 tc.tile_pool(name="ps_w", bufs=2, space="PSUM") as ps_w:

                def s_pair(s, h, a, PT_h):
                    """Two score matmuls into one 2-bank PSUM tile + one exp."""
                    hc, hp = h // HPC, D * (h % HPC)
                    kc0 = 2 * a
                    pm2 = ps_s.tile([P, 2 * SL], dt.float32, tag="sp2")
                    for idx in (0, 1):
                        kc = kc0 + idx
                        c0 = max(kc - s * DBLK, 0) * P
                        nc.tensor.matmul(
                            pm2[:, idx * SL + c0 : (idx + 1) * SL],
                            KT[hp : hp + D, hc, kc * P : (kc + 1) * P],
                            QT[hp : hp + D, hc,
                               s * SL + c0 : (s + 1) * SL],
                            start=True, stop=True)
                    c0e = max(kc0 - s * DBLK, 0) * P
                    ptf = PT_h[:].rearrange("p a b -> p (a b)")
                    nc.scalar.activation(
                        ptf[:, kc0 * SL + c0e : (kc0 + 2) * SL],
                        pm2[:, c0e:], AF.Exp, scale=inv_sqrt_d)
                    for idx in (0, 1):
                        kc = kc0 + idx
                        j = kc - s * DBLK
                        if j >= 0:
                            nc.vector.tensor_tensor(
                                PT_h[:, kc, j * P : (j + 1) * P],
                                PT_h[:, kc, j * P : (j + 1) * P],
                                tri_sb[:], OP.mult)

                def av_chunk(sp, hp_, po, PT_p, kc, kcm):
                    c0 = max(kc - sp * DBLK, 0) * P
                    nc.tensor.matmul(po[:, c0:],
                                     V[:, kc, hp_, :], PT_p[:, kc, c0:],
                                     start=(kc == 0), stop=(kc == kcm - 1),
                                     skip_group_check=True)

                def den_yt(sp, hp_, po):
                    hc, hpp = hp_ // HPC, D * (hp_ % HPC)
                    d64 = pool_rec.tile([D, SL], dt.float32, tag="d64")
                    nc.vector.tensor_copy(d64[:], po[D : 2 * D, :])
                    r64 = pool_rec.tile([D, SL], dt.float32, tag="r64")
                    nc.vector.reciprocal_approx_fast(r64[:], d64[:])
                    nc.vector.tensor_tensor(
                        YT[hpp : hpp + D, hc, sp * SL : (sp + 1) * SL],
                        po[:D, :], r64[:], OP.mult)

                def wo_slice(s, order=None, blkmap=None):
                    for tis in (order or range(TPS)):
                        ti = s * TPS + tis
                        blk = blkmap.get(ti, ti) if blkmap else ti
                        r_sb = pool_rs.tile([P, C], dt.bfloat16, tag="rsb")
                        for cs in range(NCS):
                            pm = ps_w.tile([P, CSW], dt.float32, tag="wop")
                            for ci in range(NQH):
                                nc.tensor.matmul(
                                    pm[:],
                                    YT[:, ci, ti * P : (ti + 1) * P],
                                    wo_sb[:, ci, cs * CSW : (cs + 1) * CSW],
                                    start=(ci == 0), stop=(ci == NQH - 1))
                            nc.vector.tensor_copy(
                                r_sb[:, cs * CSW : (cs + 1) * CSW], pm[:])
                        nc.sync.dma_start(rb_r[:, blk, :], r_sb[:])

                def rs_slice(s):
                    nc.gpsimd.collective_compute(
                        "ReduceScatter", OP.add, replica_groups=groups,
                        ins=[r_bounce[s * SL : (s + 1) * SL, :].opt()],
                        outs=[r_own_b[s * HS2 : (s + 1) * HS2, :].opt()])

                def rs_half(s, q):
                    # 256-row RS over a contiguous bounce range; wo_slice's
                    # blkmap placed each core's own rows in the right half
                    nc.gpsimd.collective_compute(
                        "ReduceScatter", OP.add, replica_groups=groups,
                        ins=[r_bounce[s * SL + q * 2 * P :
                                      s * SL + (q + 1) * 2 * P, :].opt()],
                        outs=[r_own_b[s * HS2 + q * P :
                                      s * HS2 + (q + 1) * P, :].opt()])

                pend = None   # (s, h, PT_h, kcm) of the head awaiting AV
                for s in range(NSL):
                    kcm = (s + 1) * DBLK
                    for h in range(HH):
                        PT_h = pool_pt.tile([P, NT, SL], dt.bfloat16,
                                            tag="PT")
                        npairs = kcm // 2
                        po = None
                        av_i = 0
                        if pend is not None:
                            ps_, ph_, pPT, pkcm = pend
                            per = -(-pkcm // npairs)
                        for a in range(npairs):
                            s_pair(s, h, a, PT_h)
                            if pend is not None:
                                for _ in range(per):
                                    if av_i < pkcm:
                                        if po is None:
                                            po = ps_o.tile([P, SL],
                                                           dt.float32,
                                                           tag="op")
                                        av_chunk(ps_, ph_, po, pPT,
                                                 av_i, pkcm)
                                        av_i += 1
                        if pend is not None:
                            while av_i < pkcm:
                                av_chunk(ps_, ph_, po, pPT, av_i, pkcm)
                                av_i += 1
                            den_yt(ps_, ph_, po)
                        pend = (s, h, PT_h, kcm)
                        if h == 0 and s > 0:
                            wo_slice(s - 1)
                        if h == 3 and s > 0:
                            rs_slice(s - 1)
                        # overlap phase-4 residual+LN2 into the last slice
                        if s == NSL - 1:
                            if h < NT2 - 2:
                                ph4_pre(h)
                            if h >= 2:
                                ph4_main(h - 2)
                # drain last head + last slice W_o / RS
                ps_, ph_, pPT, pkcm = pend
                po = ps_o.tile([P, SL], dt.float32, tag="op")
                for kc in range(pkcm):
                    av_chunk(ps_, ph_, po, pPT, kc, pkcm)
                den_yt(ps_, ph_, po)
                t0 = (NSL - 1) * TPS
                wo_slice(NSL - 1, order=(0, 2, 1, 3),
                         blkmap={t0: t0, t0 + 2: t0 + 1,
                                 t0 + 1: t0 + 2, t0 + 3: t0 + 3})
                rs_half(NSL - 1, 0)
                rs_half(NSL - 1, 1)

        # ===== phase 4 tail + 5 =====
        with tc.tile_pool(name="pht", bufs=1) as pool_ht:
            HT = pool_ht.tile([P, NF, T2], dt.float8e4)

            with ExitStack() as es_z2t:
                pool_z2t = es_z2t.enter_context(
                    tc.tile_pool(name="pz2t", bufs=1))
                Z2T = pool_z2t.tile([P, NC, T2], dt.float8e4)
                pool_wfc = es_z2t.enter_context(
                    tc.tile_pool(name="pwfc", bufs=3))
                ps_h = es_z2t.enter_context(
                    tc.tile_pool(name="ps_h", bufs=4, space="PSUM"))
                ps_trb = es_z2t.enter_context(
                    tc.tile_pool(name="ps_trb", bufs=2, space="PSUM"))

                # chunks 6,7 (needed RS of the last slice)
                for i in (NT2 - 2, NT2 - 1):
                    ph4_pre(i)
                for i in (NT2 - 2, NT2 - 1):
                    ph4_main(i)

                for i in range(NT2):
                    for jj in range(NC // NB):
                        pt = ps_trb.tile([P, NB * P], dt.bfloat16,
                                         tag="trp")
                        for j4 in range(NB):
                            j = jj * NB + j4
                            nc.tensor.transpose(
                                pt[:, j4 * P : (j4 + 1) * P],
                                Z2C[:, i, j * P : (j + 1) * P], id_sb[:])
                        nc.vector.tensor_copy(
                            Z2T[:, jj * NB : (jj + 1) * NB,
                                i * P : (i + 1) * P],
                            pt[:].rearrange("p (a b) -> p a b", a=NB))
                    # FC + gelu (fp8 DoubleRow) for the finished 512-row slice
                    if i % 4 == 3:
                        ts_ = i // 4
                        tsl = slice(ts_ * TS2, (ts_ + 1) * TS2)
                        for fo in range(FO):
                            wfc_sb = pool_wfc.tile([P, NC, FCW],
                                                   dt.float8e4, tag="wfc")
                            nc.sync.dma_start(wfc_sb[:], wfc_r[:, fo])
                            for f in range(FCW // P):
                                fg = fo * (FCW // P) + f
                                pm = ps_h.tile([P, TS2], dt.float32,
                                               tag="hp")
                                for j in range(NC // 2):
                                    nc.tensor.matmul(
                                        pm[:],
                                        wfc_sb[:, 2 * j : 2 * j + 2,
                                               f * P : (f + 1) * P],
                                        Z2T[:, 2 * j : 2 * j + 2, tsl],
                                        start=(j == 0),
                                        stop=(j == NC // 2 - 1),
                                        perf_mode=PM.DoubleRow)
                                nc.scalar.activation(
                                    HT[:, fg, tsl],
                                    pm[:], gelu_af, scale=1.0 / 16,
                                    bias=bfc_sb[:, fg : fg + 1])
                es_z2t.close()

            # phase 5b: W_out (fp8 DoubleRow) + residual
            with tc.tile_pool(name="pwout", bufs=4) as pool_wout, \
                 tc.tile_pool(name="pout", bufs=3) as pool_out, \
                 tc.tile_pool(name="ps_out", bufs=1,
                              space="PSUM") as ps_out:
                for cs in range(NCS):
                    pms = [ps_out.tile([P, CSW], dt.float32,
                                       tag=f"outp{ti}",
                                       name=f"outp_{cs}_{ti}")
                           for ti in range(NT2)]
                    for g in range(NG):
                        wout_sb = pool_wout.tile([P, 2, CSW], dt.float8e4,
                                                 tag="wout")
                        nc.sync.dma_start(
                            wout_sb[:],
                            wout_r[:, g, :, cs * CSW : (cs + 1) * CSW])
                        for ti in range(NT2):
                            nc.tensor.matmul(
                                pms[ti][:],
                                HT[:, 2 * g : 2 * g + 2,
                                   ti * P : (ti + 1) * P],
                                wout_sb[:],
                                start=(g == 0), stop=(g == NG - 1),
                                perf_mode=PM.DoubleRow)
                    for ti in range(NT2):
                        o_sb = pool_out.tile([P, CSW], dt.float32,
                                             tag="osb")
                        nc.vector.scalar_tensor_tensor(
                            o_sb[:], pms[ti][:], 1.0 / 64,
                            X2[:, ti, cs * CSW : (cs + 1) * CSW],
                            OP.mult, OP.add)
                        nc.sync.dma_start(
                            out_r[:, ti, cs * CSW : (cs + 1) * CSW],
                            o_sb[:])

    nc.compile()
    return nc


def _prep_core_inputs(b, parity, x, ln1_w, ln1_b, w_qkv, b_qkv, w_o, b_o,
                      ln2_w, ln2_b, w_fc, b_fc, w_out, b_out,
                      T_, C_, H_, D_):
    """Host-side per-core input dict (weights LN-folded + swizzled)."""
    bf16 = ml_dtypes.bfloat16
    fp8 = ml_dtypes.float8_e4m3
    HH = H_ // 2
    QH = HH * D_
    NC_ = C_ // P
    FF_ = w_fc.shape[1]
    NF_ = FF_ // P
    NG_ = NF_ // 2
    FCW_ = min(512, FF_)
    FO_ = FF_ // FCW_
    wq_eff = (ln1_w[:, None] * w_qkv).astype(np.float32)
    bq_eff = (b_qkv + ln1_b @ w_qkv).astype(np.float32)
    wfc_eff = (ln2_w[:, None] * w_fc).astype(np.float32)
    bfc_eff = (b_fc + ln2_b @ w_fc).astype(np.float32)

    def swiz_k(w):
        # [C, O] -> [P, NC, O] (partition-contiguous k-chunks), flattened
        O = w.shape[1]
        return np.ascontiguousarray(
            w.reshape(NC_, P, O).transpose(1, 0, 2).reshape(P, NC_ * O))

    h0 = parity * QH
    sl_q = slice(h0, h0 + QH)
    sl_k = slice(C_ + h0, C_ + h0 + QH)
    sl_v = slice(2 * C_ + h0, 2 * C_ + h0 + QH)
    tri = np.tril(np.ones((P, P), np.float32)).T  # tri[k,q] = 1 if k <= q
    ident = np.eye(P, dtype=np.float32)
    SL_ = min(512, T_)
    HS = SL_ // 2
    own_rows = np.concatenate([
        np.arange(s * SL_ + parity * HS, s * SL_ + (parity + 1) * HS)
        for s in range(T_ // SL_)])
    # wfc: [P, FO, NC, FCW] so each fo-chunk DMA is partition-contiguous
    wfc8 = (wfc_eff * 16).astype(fp8)
    wfc_sw = (wfc8.reshape(NC_, P, FO_, FCW_)
              .transpose(1, 2, 0, 3).reshape(P, FO_ * NC_ * FCW_))
    # wout: [P, NG, 2, C] DoubleRow pair layout
    wout8 = (w_out * 64).astype(fp8)
    wout_sw = (wout8.reshape(NG_, 2, P, C_)
               .transpose(2, 0, 1, 3).reshape(P, NG_ * 2 * C_))
    return {
        "x_full": np.ascontiguousarray(x[b]),
        # b_o is folded into the attention residual here
        "x_own": np.ascontiguousarray(x[b, own_rows] + b_o[None, :]),
        "wq": swiz_k(wq_eff[:, sl_q].astype(bf16)),
        "wk": swiz_k(wq_eff[:, sl_k].astype(bf16)),
        "wv": swiz_k(wq_eff[:, sl_v].astype(bf16)),
        "bq": np.ascontiguousarray(bq_eff[sl_q]),
        "bk": np.ascontiguousarray(bq_eff[sl_k]),
        "bv": np.ascontiguousarray(bq_eff[sl_v]),
        "wo": np.ascontiguousarray(
            w_o[h0 : h0 + QH, :].astype(bf16).reshape(QH // P, P, C_)
            .transpose(1, 0, 2).reshape(P, (QH // P) * C_)),
        "wfc": np.ascontiguousarray(wfc_sw),
        "bfc": np.ascontiguousarray(bfc_eff),
        "wout": np.ascontiguousarray(wout_sw),
        "bout": np.ascontiguousarray(b_out),
        "tri": tri.astype(bf16),
        "ident": ident.astype(bf16),
        "ident8": ident.astype(fp8),
    }


def kernel(x, ln1_w, ln1_b, w_qkv, b_qkv, w_o, b_o, ln2_w, ln2_b,
           w_fc, b_fc, w_out, b_out):
    from concourse.bass_utils import run_bass_kernel_spmd

    key = (T, C, H, D, FF, N_CORES)
    if key not in _CACHE:
        groups = [[2 * i, 2 * i + 1] for i in range(N_CORES // 2)]
        _CACHE[key] = _build(T, C, H, D, FF, N_CORES, groups)
    nc = _CACHE[key]

    args = (np.asarray(x, np.float32), np.asarray(ln1_w, np.float32),
            np.asarray(ln1_b, np.float32), np.asarray(w_qkv, np.float32),
            np.asarray(b_qkv, np.float32), np.asarray(w_o, np.float32),
            np.asarray(b_o, np.float32), np.asarray(ln2_w, np.float32),
            np.asarray(ln2_b, np.float32), np.asarray(w_fc, np.float32),
            np.asarray(b_fc, np.float32), np.asarray(w_out, np.float32),
            np.asarray(b_out, np.float32))
    in_maps = []
    for core in range(N_CORES):
        b, parity = core // 2, core % 2
        in_maps.append(_prep_core_inputs(b, parity, *args, T, C, H, D))

    global LAST_RESULT
    res = run_bass_kernel_spmd(nc, in_maps, core_ids=list(range(N_CORES)))
    LAST_RESULT = res

    SL_ = min(512, T)
    HS = SL_ // 2
    full = np.empty((B, T, C), np.float32)
    for core in range(N_CORES):
        b, parity = core // 2, core % 2
        o = res.results[core]["out"]
        for s in range(T // SL_):
            full[b, s * SL_ + parity * HS : s * SL_ + (parity + 1) * HS] = \
                o[s * HS : (s + 1) * HS]
    return full
